# revision 10
# baseline (speedup 1.0000x reference)
"""Trainium2 Bass kernel for the Bolt 64-QAM demapper MLP forward pass.

Problem: llr = (relu(relu(z @ W1 + b1) @ W2 + b2) @ W3 + b3).reshape(B, S*6)
  z [4096, 512, 3] f32, W1 [3,128], W2 [128,128], W3 [128,6].

Strategy: pure data parallel over 8 NeuronCores (batch split). All layout
transposes happen on the host (numpy); the device runs a feature-major
PE pipeline with minimal per-element work on the PSUM-evacuation engines
(the true bottleneck: only ACT and DVE can read PSUM, ~1 elem/lane/cycle).

Per core: 262144 rows in 256 tiles of 1024 rows. Row mapping within core:
  r = t*1024 + a*256 + n   (t = tile, a = partition-strip quarter, n < 256)

Device dataflow per tile (all matmul I/O bf16, fp32 PSUM):
  L1: 4 row-packed K=3 matmuls (tile_position (32a,0)), N=256 -> h1_ps
      [128,1024]; host staged zT so coords sit on partitions 32a+u.
  ACT evac: relu + b1 bias -> h1_sb bf16 (996 ns/tile, the ACT lane).
  L2: 2 K=128 N=512 matmuls (weights stay loaded) -> h2_ps [128,1024].
  DVE evac: tensor_scalar add-b2/max-0 -> h2_sb bf16 (1192 ns/tile).
  L3: 4 col-packed K=128 M=32 (W3 zero-padded) matmuls -> quarter of a
      [128,1024] out_ps shared by 4 tiles; ACT copy evac every 4 tiles.
  Output leaves feature-major ([strip, o, rows]); host re-transposes and
  adds b3.

PSUM budget (8 banks): h1 2 (single-buffered) + h2 2x2 (double) + out 2.
PE stays warm (HAM K=8/8) because each weight set's matmuls issue as
dense back-to-back groups; redundant LDWEIGHTS hide in the background
weight buffer (measured: 216 ns/MM at N=512, packed groups concurrent).
"""
import os
import numpy as np
from contextlib import ExitStack

import concourse.bacc as bacc
import concourse.mybir as mybir
import concourse.tile as tile
from concourse import bass_utils
from bass_rust import add_dep_helper

F32 = mybir.dt.float32
BF16 = mybir.dt.bfloat16
AF = mybir.ActivationFunctionType
ALU = mybir.AluOpType

N_CORES = 8
B, S, H, NB = 4096, 512, 128, 6
ROWS_TOTAL = B * S                    # 2097152
ROWS_CORE = ROWS_TOTAL // N_CORES     # 262144
TROWS = 1024                          # rows per tile
NT = ROWS_CORE // TROWS               # 256 tiles
GT = 8                                # tiles per input/output DMA group
NG = NT // GT                         # 32 DMA groups

LAST_RESULTS = None  # stashed BassKernelResults for test harness inspection


def _build_nc():
    nc = bacc.Bacc("TRN2", target_bir_lowering=False, debug=False, num_devices=N_CORES)
    # host-staged transposed input: [g, a2, u, j, n] -> [NG*6, GT*512] bf16
    # (2 strips of 512 rows per tile: concurrent row-packed matmuls must
    # each target a distinct PSUM bank, so L1 is 2 MMs x N=512, one bank each)
    zt_d = nc.dram_tensor("zt", [NG * 6, GT * 512], BF16, kind="ExternalInput")
    w1rep_d = nc.dram_tensor("w1rep", [128, H], BF16, kind="ExternalInput")
    b1_d = nc.dram_tensor("b1", [H, 1], F32, kind="ExternalInput")
    w2_d = nc.dram_tensor("w2", [H, H], BF16, kind="ExternalInput")
    b2_d = nc.dram_tensor("b2", [H, 1], F32, kind="ExternalInput")
    w3_d = nc.dram_tensor("w3", [H, 32], BF16, kind="ExternalInput")
    # feature-major output: [g, a, o(6 of 32? -> 6), j*256+n] f32
    out_d = nc.dram_tensor("out", [NG * 24, GT * 256], F32, kind="ExternalOutput")

    with tile.TileContext(nc) as tc, ExitStack() as ctx:
        const = ctx.enter_context(tc.tile_pool(name="const", bufs=1))
        zp = ctx.enter_context(tc.tile_pool(name="zp", bufs=2))
        h1p = ctx.enter_context(tc.tile_pool(name="h1p", bufs=3))
        h2p = ctx.enter_context(tc.tile_pool(name="h2p", bufs=3))
        op = ctx.enter_context(tc.tile_pool(name="op", bufs=2))
        ps_h1 = ctx.enter_context(tc.tile_pool(name="ps_h1", bufs=1, space="PSUM"))
        ps_h2 = ctx.enter_context(tc.tile_pool(name="ps_h2", bufs=2, space="PSUM"))
        ps_o = ctx.enter_context(tc.tile_pool(name="ps_o", bufs=1, space="PSUM"))

        w1rep = const.tile([128, H], BF16)
        nc.sync.dma_start(w1rep[:], w1rep_d.ap())
        w2sb = const.tile([H, H], BF16)
        nc.sync.dma_start(w2sb[:], w2_d.ap())
        w3sb = const.tile([H, 32], BF16)
        nc.sync.dma_start(w3sb[:], w3_d.ap())
        b1sb = const.tile([H, 1], F32)
        nc.sync.dma_start(b1sb[:], b1_d.ap())
        b2sb = const.tile([H, 1], F32)
        nc.sync.dma_start(b2sb[:], b2_d.ap())

        zt_v = zt_d.ap().rearrange("(g x) n -> g x n", g=NG)       # [NG, 6, GT*512]
        out_v = out_d.ap().rearrange("(g x) n -> g x n", g=NG)     # [NG, 24, 2048]

        # Keep matmul groups adjacent in the PE stream (same-weight bursts
        # stream at N-cycles; avoids scheduler interleaving weight sets).
        last_mm = [None]

        def mm(*args, **kw):
            inst = nc.tensor.matmul(*args, **kw)
            if last_mm[0] is not None:
                add_dep_helper(inst.ins, last_mm[0].ins, False, "pe order")
            last_mm[0] = inst
            return inst

        zins, h1pss, h1s, h2pss, h2s, opss, outsbs = {}, {}, {}, {}, {}, {}, {}

        def stage_in(t):
            g, j = divmod(t, GT)
            if j == 0:
                zin = zp.tile([128, GT * 512], BF16, tag="zin", name="zin")
                src = zt_v[g].rearrange("(a u) n -> a u n", a=2)   # [2, 3, GT*512]
                for a in range(2):
                    nc.sync.dma_start(zin[32 * a: 32 * a + 3, :], src[a])
                zins[g] = zin

        def stage_l1(t):
            g, j = divmod(t, GT)
            zin = zins[g]
            h1_ps = ps_h1.tile([128, TROWS], F32, tag="h1", name="h1ps")
            for a in range(2):
                mm(
                    h1_ps[:, a * 512: (a + 1) * 512],
                    w1rep[32 * a: 32 * a + 3, :],
                    zin[32 * a: 32 * a + 3, j * 512: (j + 1) * 512],
                    tile_position=(32 * a, 0),
                )
            h1pss[t] = h1_ps

        def evac_h1(t):
            h1_ps = h1pss.pop(t)
            h1_sb = h1p.tile([128, TROWS], BF16, tag="h1", name="h1sb")
            nc.scalar.activation(h1_sb[:], h1_ps[:], AF.Relu, bias=b1sb[:])
            h1s[t] = h1_sb

        def stage_l2(t):
            h1_sb = h1s.pop(t)
            h2_ps = ps_h2.tile([128, TROWS], F32, tag="h2", name="h2ps")
            for k in range(2):
                mm(
                    h2_ps[:, k * 512: (k + 1) * 512],
                    w2sb[:],
                    h1_sb[:, k * 512: (k + 1) * 512],
                )
            h2pss[t] = h2_ps

        def evac_h2(t):
            h2_ps = h2pss.pop(t)
            h2_sb = h2p.tile([128, TROWS], BF16, tag="h2", name="h2sb")
            nc.vector.tensor_scalar(
                h2_sb[:], h2_ps[:], b2sb[:], 0.0, op0=ALU.add, op1=ALU.max
            )
            h2s[t] = h2_sb

        def stage_l3(t):
            q = t % 4
            h2_sb = h2s.pop(t)
            if q == 0:
                opss[t // 4] = ps_o.tile([128, 1024], F32, tag="o", name="ops")
            out_ps = opss[t // 4]
            for a in range(4):
                mm(
                    out_ps[32 * a: 32 * a + 32, q * 256: (q + 1) * 256],
                    w3sb[:],
                    h2_sb[:, a * 256: (a + 1) * 256],
                    tile_position=(0, 32 * a),
                )
            if q == 3:
                evac_out(t // 4)

        def evac_out(grp4):
            # grp4 covers tiles 4*grp4 .. 4*grp4+3; two grp4s share one out_sb
            out_ps = opss.pop(grp4)
            g, half = divmod(grp4, 2)
            if half == 0:
                outsbs[g] = op.tile([128, GT * 256], F32, tag="osb", name="osb")
            out_sb = outsbs[g]
            nc.scalar.copy(out_sb[:, half * 1024: (half + 1) * 1024], out_ps[:])
            if half == 1:
                out_sb = outsbs.pop(g)
                dst = out_v[g].rearrange("(a o) n -> a o n", a=4)
                for a in range(4):
                    nc.sync.dma_start(dst[a], out_sb[32 * a: 32 * a + 6, :])

        stage_in(0)
        for t in range(NT + 2):
            if t + GT < NT:
                stage_in(t + GT)
            if 1 <= t <= NT:
                stage_l2(t - 1)
            if t >= 2:
                stage_l3(t - 2)
            if t < NT:
                stage_l1(t)
                evac_h1(t)
            if 1 <= t <= NT:
                evac_h2(t - 1)

    nc.compile()
    return nc


def kernel(z, W1, b1, W2, b2, W3, b3):
    global LAST_RESULTS
    z = np.asarray(z, dtype=np.float32)
    W1 = np.asarray(W1, dtype=np.float32)
    b1 = np.asarray(b1, dtype=np.float32)
    W2 = np.asarray(W2, dtype=np.float32)
    b2 = np.asarray(b2, dtype=np.float32)
    W3 = np.asarray(W3, dtype=np.float32)
    b3 = np.asarray(b3, dtype=np.float32)

    bfnp = mybir.dt.np(BF16)
    # host-side weight prep (tiny)
    w1rep = np.zeros((128, H), bfnp)
    for a in range(4):
        w1rep[32 * a: 32 * a + 3] = W1.astype(bfnp)
    w3pad = np.zeros((H, 32), bfnp)
    w3pad[:, :NB] = W3.astype(bfnp)

    # host-side input staging: transpose to [g, a, u, j, n] per core
    z_rows = np.ascontiguousarray(z).reshape(ROWS_TOTAL, 3)
    shards = np.split(z_rows, N_CORES, axis=0)

    common = {
        "w1rep": w1rep,
        "b1": np.ascontiguousarray(b1.reshape(H, 1)),
        "w2": np.ascontiguousarray(W2.astype(bfnp)),
        "b2": np.ascontiguousarray(b2.reshape(H, 1)),
        "w3": w3pad,
    }
    in_maps = []
    for s in shards:
        zc = s.reshape(NG, GT, 2, 512, 3)          # [g, j, a2, n, u]
        zt = zc.transpose(0, 2, 4, 1, 3)           # [g, a2, u, j, n]
        zt = np.ascontiguousarray(zt.astype(bfnp)).reshape(NG * 6, GT * 512)
        in_maps.append(dict(common, zt=zt))

    nc = _build_nc()
    res = bass_utils.run_bass_kernel_spmd(
        nc,
        in_maps,
        core_ids=list(range(N_CORES)),
        trace=bool(os.environ.get("KBENCH_TRACE")),
    )
    LAST_RESULTS = res

    # host-side output reassembly: [g, a, o, j, n] -> rows x 6, + b3
    outs = []
    for i in range(N_CORES):
        arr = res.results[i]["out"].reshape(NG, 4, NB, GT, 256)
        arr = arr.transpose(0, 3, 1, 4, 2)         # [g, j, a, n, o]
        outs.append(arr.reshape(ROWS_CORE, NB))
    full = np.concatenate(outs, axis=0) + b3[None, :]
    return full.reshape(B, S * NB).astype(np.float32)


# revision 11
# speedup vs baseline: 1.5481x; 1.5481x over previous
"""Trainium2 Bass kernel for the Bolt 64-QAM demapper MLP forward pass.

Problem: llr = (relu(relu(z @ W1 + b1) @ W2 + b2) @ W3 + b3).reshape(B, S*6)
  z [4096, 512, 3] f32, W1 [3,128], W2 [128,128], W3 [128,6].

Strategy: pure data parallel over 8 NeuronCores (batch split). All layout
transposes happen on the host (numpy); the device runs a feature-major
PE pipeline with minimal per-element work on the PSUM-evacuation engines
(the true bottleneck: only ACT and DVE can read PSUM, ~1 elem/lane/cycle).

Per core: 262144 rows in 256 tiles of 1024 rows. Row mapping within core:
  r = t*1024 + a*256 + n   (t = tile, a = partition-strip quarter, n < 256)

Device dataflow per tile (all matmul I/O bf16, fp32 PSUM):
  L1: 4 row-packed K=3 matmuls (tile_position (32a,0)), N=256 -> h1_ps
      [128,1024]; host staged zT so coords sit on partitions 32a+u.
  ACT evac: relu + b1 bias -> h1_sb bf16 (996 ns/tile, the ACT lane).
  L2: 2 K=128 N=512 matmuls (weights stay loaded) -> h2_ps [128,1024].
  DVE evac: tensor_scalar add-b2/max-0 -> h2_sb bf16 (1192 ns/tile).
  L3: 4 col-packed K=128 M=32 (W3 zero-padded) matmuls -> quarter of a
      [128,1024] out_ps shared by 4 tiles; ACT copy evac every 4 tiles.
  Output leaves feature-major ([strip, o, rows]); host re-transposes and
  adds b3.

PSUM budget (8 banks): h1 2 (single-buffered) + h2 2x2 (double) + out 2.
PE stays warm (HAM K=8/8) because each weight set's matmuls issue as
dense back-to-back groups; redundant LDWEIGHTS hide in the background
weight buffer (measured: 216 ns/MM at N=512, packed groups concurrent).
"""
import os
import numpy as np
from contextlib import ExitStack

import concourse.bacc as bacc
import concourse.mybir as mybir
import concourse.tile as tile
from concourse import bass_utils
from bass_rust import add_dep_helper

F32 = mybir.dt.float32
BF16 = mybir.dt.bfloat16
AF = mybir.ActivationFunctionType
ALU = mybir.AluOpType

N_CORES = 8
B, S, H, NB = 4096, 512, 128, 6
ROWS_TOTAL = B * S                    # 2097152
ROWS_CORE = ROWS_TOTAL // N_CORES     # 262144
TROWS = 1024                          # rows per tile
NT = ROWS_CORE // TROWS               # 256 tiles
GT = 8                                # tiles per input/output DMA group
NG = NT // GT                         # 32 DMA groups

LAST_RESULTS = None  # stashed BassKernelResults for test harness inspection


def _build_nc():
    nc = bacc.Bacc("TRN2", target_bir_lowering=False, debug=False, num_devices=N_CORES)
    # host-staged transposed input: [g, a2, u, j, n] -> [NG*6, GT*512] bf16
    # (2 strips of 512 rows per tile: concurrent row-packed matmuls must
    # each target a distinct PSUM bank, so L1 is 2 MMs x N=512, one bank each)
    zt_d = nc.dram_tensor("zt", [NG * 6, GT * 512], BF16, kind="ExternalInput")
    w1rep_d = nc.dram_tensor("w1rep", [128, H], BF16, kind="ExternalInput")
    b1_d = nc.dram_tensor("b1", [H, 1], F32, kind="ExternalInput")
    w2_d = nc.dram_tensor("w2", [H, H], BF16, kind="ExternalInput")
    b2_d = nc.dram_tensor("b2", [H, 1], F32, kind="ExternalInput")
    w3_d = nc.dram_tensor("w3", [H, 32], BF16, kind="ExternalInput")
    # feature-major output: [g, a, o(6 of 32? -> 6), j*256+n] f32
    out_d = nc.dram_tensor("out", [NG * 24, GT * 256], F32, kind="ExternalOutput")

    with tile.TileContext(nc) as tc, ExitStack() as ctx:
        const = ctx.enter_context(tc.tile_pool(name="const", bufs=1))
        zp = ctx.enter_context(tc.tile_pool(name="zp", bufs=2))
        h1p = ctx.enter_context(tc.tile_pool(name="h1p", bufs=3))
        h2p = ctx.enter_context(tc.tile_pool(name="h2p", bufs=3))
        op = ctx.enter_context(tc.tile_pool(name="op", bufs=2))
        ps_h1 = ctx.enter_context(tc.tile_pool(name="ps_h1", bufs=1, space="PSUM"))
        ps_h2 = ctx.enter_context(tc.tile_pool(name="ps_h2", bufs=2, space="PSUM"))
        ps_o = ctx.enter_context(tc.tile_pool(name="ps_o", bufs=1, space="PSUM"))

        w1rep = const.tile([128, H], BF16)
        nc.sync.dma_start(w1rep[:], w1rep_d.ap())
        w2sb = const.tile([H, H], BF16)
        nc.sync.dma_start(w2sb[:], w2_d.ap())
        w3sb = const.tile([H, 32], BF16)
        nc.sync.dma_start(w3sb[:], w3_d.ap())
        b1sb = const.tile([H, 1], F32)
        nc.sync.dma_start(b1sb[:], b1_d.ap())
        b2sb = const.tile([H, 1], F32)
        nc.sync.dma_start(b2sb[:], b2_d.ap())

        zt_v = zt_d.ap().rearrange("(g x) n -> g x n", g=NG)       # [NG, 6, GT*512]
        out_v = out_d.ap().rearrange("(g x) n -> g x n", g=NG)     # [NG, 24, 2048]

        # Keep matmul groups adjacent in the PE stream (same-weight bursts
        # stream at N-cycles; avoids scheduler interleaving weight sets).
        last_mm = [None]

        def mm(*args, **kw):
            inst = nc.tensor.matmul(*args, **kw)
            if last_mm[0] is not None:
                add_dep_helper(inst.ins, last_mm[0].ins, False, "pe order")
            last_mm[0] = inst
            return inst

        zins, h1pss, h1s, h2pss, h2s, opss, outsbs = {}, {}, {}, {}, {}, {}, {}

        def stage_in(t):
            g, j = divmod(t, GT)
            if j == 0:
                zin = zp.tile([128, GT * 512], BF16, tag="zin", name="zin")
                src = zt_v[g].rearrange("(a u) n -> a u n", a=2)   # [2, 3, GT*512]
                for a in range(2):
                    nc.sync.dma_start(zin[32 * a: 32 * a + 3, :], src[a])
                zins[g] = zin

        def stage_l1(t):
            g, j = divmod(t, GT)
            zin = zins[g]
            h1_ps = ps_h1.tile([128, TROWS], F32, tag="h1", name="h1ps")
            for a in range(2):
                mm(
                    h1_ps[:, a * 512: (a + 1) * 512],
                    w1rep[32 * a: 32 * a + 3, :],
                    zin[32 * a: 32 * a + 3, j * 512: (j + 1) * 512],
                    tile_position=(32 * a, 0),
                )
            h1pss[t] = h1_ps

        def evac_h1(t):
            h1_ps = h1pss.pop(t)
            h1_sb = h1p.tile([128, TROWS], BF16, tag="h1", name="h1sb")
            nc.scalar.activation(h1_sb[:], h1_ps[:], AF.Relu, bias=b1sb[:])
            h1s[t] = h1_sb

        def stage_l2(t):
            h1_sb = h1s.pop(t)
            h2_ps = ps_h2.tile([128, TROWS], F32, tag="h2", name="h2ps")
            for k in range(2):
                mm(
                    h2_ps[:, k * 512: (k + 1) * 512],
                    w2sb[:],
                    h1_sb[:, k * 512: (k + 1) * 512],
                )
            h2pss[t] = h2_ps

        def evac_h2(t):
            h2_ps = h2pss.pop(t)
            h2_sb = h2p.tile([128, TROWS], BF16, tag="h2", name="h2sb")
            nc.vector.tensor_scalar(
                h2_sb[:], h2_ps[:], b2sb[:], 0.0, op0=ALU.add, op1=ALU.max
            )
            h2s[t] = h2_sb

        def stage_l3(t):
            q = t % 4
            h2_sb = h2s.pop(t)
            if q == 0:
                opss[t // 4] = ps_o.tile([128, 1024], F32, tag="o", name="ops")
            out_ps = opss[t // 4]
            for a in range(4):
                mm(
                    out_ps[32 * a: 32 * a + 32, q * 256: (q + 1) * 256],
                    w3sb[:],
                    h2_sb[:, a * 256: (a + 1) * 256],
                    tile_position=(0, 32 * a),
                )
            if q == 3:
                evac_out(t // 4)

        def evac_out(grp4):
            # grp4 covers tiles 4*grp4 .. 4*grp4+3; two grp4s share one out_sb
            out_ps = opss.pop(grp4)
            g, half = divmod(grp4, 2)
            if half == 0:
                outsbs[g] = op.tile([128, GT * 256], F32, tag="osb", name="osb")
            out_sb = outsbs[g]
            nc.scalar.copy(out_sb[:, half * 1024: (half + 1) * 1024], out_ps[:])
            if half == 1:
                out_sb = outsbs.pop(g)
                dst = out_v[g].rearrange("(a o) n -> a o n", a=4)
                for a in range(4):
                    nc.sync.dma_start(dst[a], out_sb[32 * a: 32 * a + 6, :])

        # Pipeline offsets: L2 consumes h1_sb from 2 periods back and L3
        # consumes h2_sb from 3 back, so no PE instruction ever waits on an
        # evac issued in the same period (keeps PE dense -> HAM warm).
        stage_in(0)
        for t in range(NT + 3):
            if t + GT < NT:
                stage_in(t + GT)
            if 2 <= t < NT + 2:
                stage_l2(t - 2)
            if t >= 3:
                stage_l3(t - 3)
            if t < NT:
                stage_l1(t)
                evac_h1(t)
            if 2 <= t < NT + 2:
                evac_h2(t - 2)

    nc.compile()
    return nc


def kernel(z, W1, b1, W2, b2, W3, b3):
    global LAST_RESULTS
    z = np.asarray(z, dtype=np.float32)
    W1 = np.asarray(W1, dtype=np.float32)
    b1 = np.asarray(b1, dtype=np.float32)
    W2 = np.asarray(W2, dtype=np.float32)
    b2 = np.asarray(b2, dtype=np.float32)
    W3 = np.asarray(W3, dtype=np.float32)
    b3 = np.asarray(b3, dtype=np.float32)

    bfnp = mybir.dt.np(BF16)
    # host-side weight prep (tiny)
    w1rep = np.zeros((128, H), bfnp)
    for a in range(4):
        w1rep[32 * a: 32 * a + 3] = W1.astype(bfnp)
    w3pad = np.zeros((H, 32), bfnp)
    w3pad[:, :NB] = W3.astype(bfnp)

    # host-side input staging: transpose to [g, a, u, j, n] per core
    z_rows = np.ascontiguousarray(z).reshape(ROWS_TOTAL, 3)
    shards = np.split(z_rows, N_CORES, axis=0)

    common = {
        "w1rep": w1rep,
        "b1": np.ascontiguousarray(b1.reshape(H, 1)),
        "w2": np.ascontiguousarray(W2.astype(bfnp)),
        "b2": np.ascontiguousarray(b2.reshape(H, 1)),
        "w3": w3pad,
    }
    in_maps = []
    for s in shards:
        zc = s.reshape(NG, GT, 2, 512, 3)          # [g, j, a2, n, u]
        zt = zc.transpose(0, 2, 4, 1, 3)           # [g, a2, u, j, n]
        zt = np.ascontiguousarray(zt.astype(bfnp)).reshape(NG * 6, GT * 512)
        in_maps.append(dict(common, zt=zt))

    nc = _build_nc()
    res = bass_utils.run_bass_kernel_spmd(
        nc,
        in_maps,
        core_ids=list(range(N_CORES)),
        trace=bool(os.environ.get("KBENCH_TRACE")),
    )
    LAST_RESULTS = res

    # host-side output reassembly: [g, a, o, j, n] -> rows x 6, + b3
    outs = []
    for i in range(N_CORES):
        arr = res.results[i]["out"].reshape(NG, 4, NB, GT, 256)
        arr = arr.transpose(0, 3, 1, 4, 2)         # [g, j, a, n, o]
        outs.append(arr.reshape(ROWS_CORE, NB))
    full = np.concatenate(outs, axis=0) + b3[None, :]
    return full.reshape(B, S * NB).astype(np.float32)
